# revision 7
# baseline (speedup 1.0000x reference)
"""GQA attention (B=4, S=1024, H=4096, 32 q heads / 8 kv heads, rotary) on 8 trn2 cores.

Sharding: DP4 x TP2. Core c = 2*b + j handles batch b with kv-head half j:
  - column-parallel wq/wk/wv (16 q heads / 4 kv heads per core)
  - row-parallel wo -> partial [S, H] outputs, host sums core pairs.

v2 dataflow (bf16 operands, fp32 PSUM/rope math, one K pass, no DRAM spills):
  xT resident in SBUF as bf16 [128, 32, 1024]; weights streamed bf16.
  Projection blocks (24 x 128 cols) accumulate full K=4096 in PSUM,
  rope applied to an SBUF copy (DVE), pair-swap via SBUF DMA.
  Attention for q-block i is interleaved into the PE stream between
  projection blocks (scores -> ACT exp -> DVE-tree denominator reduce ->
  skinny PE sum/broadcast matmuls -> attn@v -> DVE normalize into oT).
  Phase 3: out = oT.T @ wo with wo streamed bf16.
"""

import numpy as np

B = 4
S = 1024
H = 4096
D = 128
HQ = 32
HKV = 8
G = 4
NCORES = 8
QC = 2048  # q cols per core
KC = 512  # k cols per core
VC = 512  # v cols per core
COH = 2048  # wo rows per core
ROPE_BASE = 10000.0

_CACHE = {}


def _build(reps=1, gpsimd_denom=False, hw_loop=False, staggered=False):
    import concourse.tile as tile
    from concourse import bacc, bass_isa, mybir
    from concourse.masks import make_identity

    fp32 = mybir.dt.float32
    bf16 = mybir.dt.bfloat16

    nc = bacc.Bacc(None, target_bir_lowering=False)

    # packed layouts: per-partition-contiguous so each DMA descriptor is
    # one 4-8 KiB run (256 B descriptors run far below HBM line rate)
    xT_d = nc.dram_tensor("xT", [128, H // 128, S], bf16, kind="ExternalInput")
    wq_d = nc.dram_tensor(
        "wq", [QC // 128, 128, H // 128, 128], bf16, kind="ExternalInput"
    )
    wk_d = nc.dram_tensor(
        "wk", [KC // 128, 128, H // 128, 128], bf16, kind="ExternalInput"
    )
    wv_d = nc.dram_tensor(
        "wv", [VC // 128, 128, H // 128, 128], bf16, kind="ExternalInput"
    )
    wo_d = nc.dram_tensor(
        "wo", [2, 8, 128, 8, 512], bf16, kind="ExternalInput"
    )
    aq_d = nc.dram_tensor("ropeAq", [D, S], bf16, kind="ExternalInput")
    bq_d = nc.dram_tensor("ropeBq", [D, S], bf16, kind="ExternalInput")
    ak_d = nc.dram_tensor("ropeAk", [D, S], bf16, kind="ExternalInput")
    bk_d = nc.dram_tensor("ropeBk", [D, S], bf16, kind="ExternalInput")
    out_d = nc.dram_tensor("out", [S, H], fp32, kind="ExternalOutput")

    out_r = out_d.rearrange("(tb p) h -> tb p h", p=128)  # [8, 128, 4096]

    NKO = H // 128  # 32 contraction tiles

    # projection block schedule: per kv head h: k, v, then 4 q blocks
    sched = []
    for h in range(4):
        sched.append(("k", h))
        sched.append(("v", h))
        for g in range(4):
            sched.append(("q", h * 4 + g))

    def one_rep(tc):
        with (
            tc.tile_pool(name="persist", bufs=1) as persist,
            tc.tile_pool(name="konst", bufs=1) as konst,
            tc.tile_pool(name="wopre", bufs=1) as wopre,
        ):
            oT = persist.tile([128, 16, S], bf16)  # 32 KiB/part
            ident = konst.tile([128, 128], bf16)
            make_identity(nc, ident[:])
            if not gpsimd_denom:
                f32r = mybir.dt.float32r
                ones_f = konst.tile([128, 128], fp32)
                nc.vector.memset(ones_f[:], 1.0)
                ones = konst.tile([128, 128], f32r)
                nc.vector.tensor_copy(ones[:], ones_f[:])
            wot00 = wopre.tile([128, 8, 512], bf16, name="wot00")

            with (
                tc.tile_pool(name="xt", bufs=1) as xpool,
                tc.tile_pool(name="kv", bufs=1) as kvpool,
                tc.tile_pool(name="maps", bufs=1) as mpool,
                tc.tile_pool(name="wt", bufs=3) as wpool,
                tc.tile_pool(name="qt", bufs=2) as qpool,
                tc.tile_pool(name="ev", bufs=2) as epool,
                tc.tile_pool(name="vraw", bufs=2) as vrawpool,
                tc.tile_pool(name="ex", bufs=2) as expool,
                tc.tile_pool(name="den", bufs=2) as dpool,
                tc.tile_pool(
                    name="psp", bufs=(4 if gpsimd_denom else 3), space="PSUM"
                ) as psp,
                tc.tile_pool(name="pssc", bufs=2, space="PSUM") as pssc,
                tc.tile_pool(name="pso", bufs=1, space="PSUM") as psop,
                tc.tile_pool(
                    name="pst", bufs=(1 if gpsimd_denom else 2), space="PSUM"
                ) as pst,
            ):
                # resident tensors
                xt = xpool.tile([128, NKO, S], bf16)  # 64 KiB/part
                kT = kvpool.tile([128, 4, S], bf16)  # 8 KiB/part
                v = kvpool.tile([128, 8, VC], bf16)  # 8 KiB/part

                # loads: first weight, then xt chunks (16 x 2ko), maps
                def load_wt(kind, idx, split=1):
                    w_d = {"q": wq_d, "k": wk_d, "v": wv_d}[kind]
                    wt = wpool.tile([128, NKO, 128], bf16, tag="wt", name="wt")
                    step = NKO // split
                    for s in range(split):
                        ks = slice(s * step, (s + 1) * step)
                        nc.sync.dma_start(wt[:, ks, :], w_d[idx][:, ks, :])
                    return wt

                # first block's weight split into 4 so the PE can start on
                # ko 0-7 while the rest of the cold-start loads stream in
                wt_q = [load_wt(*sched[0], split=4)]
                XCH = 2  # ko tiles per xt load chunk
                for chx in range(NKO // XCH):
                    nc.sync.dma_start(
                        xt[:, chx * XCH : (chx + 1) * XCH, :],
                        xT_d[:, chx * XCH : (chx + 1) * XCH, :],
                    )
                maps = {}
                for nm, dram in (
                    ("Aq", aq_d), ("Bq", bq_d), ("Ak", ak_d), ("Bk", bk_d)
                ):
                    mt = mpool.tile([128, S], bf16, name=nm)
                    nc.sync.dma_start(mt[:], dram[:])
                    maps[nm] = mt
                wt_q.append(load_wt(*sched[1]))

                def rope(raw, Am, Bm, out_ap):
                    """out = A*raw + B*swap(raw), raw [128, S] bf16 in SBUF."""
                    sw = epool.tile([128, S], bf16, tag="sw", name="sw")
                    nc.sync.dma_start(sw[0:64, :], raw[64:128, :])
                    nc.sync.dma_start(sw[64:128, :], raw[0:64, :])
                    t1 = epool.tile([128, S], fp32, tag="t1", name="t1", bufs=1)
                    nc.vector.tensor_mul(t1[:], raw[:], Am[:])
                    t2 = epool.tile([128, S], fp32, tag="t2", name="t2", bufs=1)
                    nc.vector.tensor_mul(t2[:], sw[:], Bm[:])
                    nc.vector.tensor_add(out_ap, t1[:], t2[:])

                def proj_begin(i):
                    """A-half (t 0:512) of projection block i."""
                    wt = wt_q.pop(0)
                    # prefetch two blocks ahead: a full block of PE time
                    # (~13.6us) of DMA lead so LDWEIGHTS never stalls
                    if i + 2 < len(sched):
                        wt_q.append(load_wt(*sched[i + 2]))
                    raw = vrawpool.tile([128, S], bf16, tag="raw", name="raw")
                    psA = psp.tile([128, 512], fp32, tag="psp", name="psA")
                    for ko in range(NKO):
                        nc.tensor.matmul(
                            psA[:], wt[:, ko, :], xt[:, ko, 0:512],
                            start=(ko == 0), stop=(ko == NKO - 1),
                        )
                    nc.scalar.copy(raw[:, 0:512], psA[:])
                    return wt, raw

                def proj_end(ctx, i):
                    """B-half + rope/v epilogue; returns (cb, qt) for q blocks."""
                    kind, idx = sched[i]
                    wt, raw = ctx
                    psB = psp.tile([128, 512], fp32, tag="psp", name="psB")
                    for ko in range(NKO):
                        nc.tensor.matmul(
                            psB[:], wt[:, ko, :], xt[:, ko, 512:1024],
                            start=(ko == 0), stop=(ko == NKO - 1),
                        )
                    nc.scalar.copy(raw[:, 512:1024], psB[:])
                    if kind == "q":
                        qt = qpool.tile([128, S], bf16, tag="qt", name="qt")
                        rope(raw, maps["Aq"], maps["Bq"], qt[:])
                        return (idx, qt)
                    if kind == "k":
                        rope(raw, maps["Ak"], maps["Bk"], kT[:, idx, :])
                        return None
                    # v: PE-transpose [d, t] -> [t, d] blocks into natural v
                    for tb in range(8):
                        pt = pst.tile([128, 128], bf16, tag="misc", name="pt")
                        nc.tensor.transpose(
                            pt[:], raw[:, tb * 128 : (tb + 1) * 128], ident[:]
                        )
                        nc.vector.tensor_copy(
                            v[:, tb, idx * 128 : (idx + 1) * 128], pt[:]
                        )
                    return None

                def attn_scores(cb, qt, sh):
                    """scores + exp for (cb, sh): returns expT tile."""
                    h = cb // 4
                    ss = slice(sh * 512, sh * 512 + 512)
                    expT = expool.tile([128, 8, 512], bf16, tag="expT", name="expT")
                    for tb in range(8):
                        psc = pssc.tile([128, 512], fp32, tag="psc", name="psc")
                        nc.tensor.matmul(
                            psc[:],
                            kT[:, h, tb * 128 : (tb + 1) * 128],
                            qt[:, ss],
                            start=True, stop=True,
                        )
                        nc.scalar.activation(
                            expT[:, tb], psc[:],
                            mybir.ActivationFunctionType.Exp,
                        )
                    return expT

                f32r = mybir.dt.float32r

                def attn_tree(expT):
                    """DVE tree: pre-reduce 8 exp tiles to one [128, 512]."""
                    acc = dpool.tile([128, 512], f32r, tag="acc", name="acc",
                                     bufs=4)
                    nc.vector.tensor_add(acc[:], expT[:, 0], expT[:, 1])
                    for tb in range(2, 8):
                        nxt = dpool.tile([128, 512], f32r, tag="acc", name="acc",
                                         bufs=4)
                        nc.vector.tensor_add(nxt[:], acc[:], expT[:, tb])
                        acc = nxt
                    return acc[:]

                def attn_pden(g):
                    """cross-partition sum of acc, broadcast to all rows."""
                    if gpsimd_denom:
                        red = dpool.tile([128, 512], fp32, tag="acc", name="red",
                                         bufs=4)
                        nc.gpsimd.partition_all_reduce(
                            red[:], g["acc"], 128, bass_isa.ReduceOp.add
                        )
                        inv = dpool.tile([128, 512], fp32, tag="inv", name="inv")
                        nc.vector.reciprocal(inv[:], red[:])
                        g["inv"] = inv[:]
                        return
                    # ones[128,128].T @ acc = column sums broadcast to all
                    # 128 partitions in a single matmul
                    pbc = pst.tile([128, 512], fp32, tag="misc", name="pbc")
                    nc.tensor.matmul(
                        pbc[:], ones[:, :], g["acc"], start=True, stop=True
                    )
                    g["den_ps"] = pbc[:]

                def attn_pbc(g):
                    """1/den straight from PSUM via fast Newton reciprocal."""
                    if gpsimd_denom:
                        return
                    inv = dpool.tile([128, 512], fp32, tag="inv", name="inv")
                    nc.vector.reciprocal_approx_fast(inv[:], g["den_ps"])
                    g["inv"] = inv[:]

                def attn_av(g):
                    cb, sh, expT = g["cb"], g["sh"], g["expT"]
                    h = cb // 4
                    ss = slice(sh * 512, sh * 512 + 512)
                    po = psop.tile([128, 512], fp32, tag="po", name="po")
                    for tb in range(8):
                        nc.tensor.matmul(
                            po[:],
                            v[:, tb, h * 128 : (h + 1) * 128],
                            expT[:, tb],
                            start=(tb == 0), stop=(tb == 7),
                        )
                    nc.vector.tensor_mul(oT[:, cb, ss], po[:], g["inv"])

                # software pipeline. Iteration i issues, in PE order:
                #   projA(i) | pden(G_{i-2}) | projB(i)+epilogue | pbc+attnv
                #   (G_{i-2}) | scores+exp+tree (G_{i-1})
                # so every cross-engine chain (exp on ACT, tree on DVE,
                # reciprocal) has a full projection block of PE time to hide.
                sc_q = []  # (cb, qt): q blocks awaiting scores
                s2_q = []  # groups awaiting pden
                for i in range(len(sched) + 2):
                    ctx = proj_begin(i) if i < len(sched) else None
                    s3_q = []
                    while s2_q:
                        g = s2_q.pop(0)
                        attn_pden(g)
                        s3_q.append(g)
                    new_q = proj_end(ctx, i) if ctx is not None else None
                    while s3_q:
                        g = s3_q.pop(0)
                        attn_pbc(g)
                        attn_av(g)
                    while sc_q:
                        cb0, qt0 = sc_q.pop(0)
                        for sh in range(2):
                            e = attn_scores(cb0, qt0, sh)
                            acc = attn_tree(e)
                            s2_q.append(
                                {"cb": cb0, "sh": sh, "expT": e, "acc": acc}
                            )
                    if new_q is not None:
                        sc_q.append(new_q)

                # prefetch first wo strip while attention tail drains
                nc.sync.dma_start(wot00[:], wo_d[0, 0])

            # ---------------- Phase 3: out = oT.T @ wo ----------------
            with (
                tc.tile_pool(name="wot", bufs=4) as wopool,
                tc.tile_pool(name="outp", bufs=3) as outpool,
                tc.tile_pool(name="psout", bufs=3, space="PSUM") as psout,
            ):
                def load_wo_strip(hh, hf):
                    wot = wopool.tile(
                        [128, 8, 512], bf16, tag="wo", name=f"wo{hf}"
                    )
                    nc.sync.dma_start(wot[:], wo_d[hf, hh])
                    return wot

                wo_next = [wot00, load_wo_strip(0, 1)]
                for hh in range(8):
                    hs = slice(hh * 512, hh * 512 + 512)
                    wotA, wotB = wo_next
                    for tb in range(8):
                        pso_ = psout.tile([128, 512], fp32, tag="pso", name="pso_")
                        for co in range(8):
                            nc.tensor.matmul(
                                pso_[:],
                                oT[:, co, tb * 128 : (tb + 1) * 128],
                                wotA[:, co, :],
                                start=(co == 0), stop=False,
                            )
                        if hh < 7:
                            if tb == 0:
                                wo_next[0] = load_wo_strip(hh + 1, 0)
                            elif tb == 1:
                                wo_next[1] = load_wo_strip(hh + 1, 1)
                        for co in range(8, 16):
                            nc.tensor.matmul(
                                pso_[:],
                                oT[:, co, tb * 128 : (tb + 1) * 128],
                                wotB[:, co - 8, :],
                                start=False, stop=(co == 15),
                            )
                        ot = outpool.tile([128, 512], fp32, tag="ot", name="ot")
                        nc.scalar.copy(ot[:], pso_[:])
                        nc.sync.dma_start(out_r[tb, :, hs], ot[:])

    with tile.TileContext(nc) as tc, nc.allow_low_precision(
        reason="bf16 matmul pipeline, fp32 accumulation"
    ):
        if hw_loop and reps > 1:
            unroll = 1
            while reps % (unroll * 2) == 0 and unroll < hw_loop:
                unroll *= 2
            with tc.For_i(0, reps // unroll, 1, staggered_reset=staggered):
                for _u in range(unroll):
                    one_rep(tc)
        else:
            for _rep in range(reps):
                one_rep(tc)

    nc.compile()
    return nc


def _to_bf16(a):
    import ml_dtypes

    return np.asarray(a, dtype=np.float32).astype(ml_dtypes.bfloat16)


def _host_prep(x, wq, wk, wv, wo, start_pos):
    x = np.asarray(x, dtype=np.float32)
    wq = np.asarray(wq, dtype=np.float32)
    wk = np.asarray(wk, dtype=np.float32)
    wv = np.asarray(wv, dtype=np.float32)
    wo = np.asarray(wo, dtype=np.float32)
    sp = int(np.asarray(start_pos))

    perm = np.concatenate([np.arange(0, 128, 2), np.arange(1, 128, 2)])

    def permute_cols(w):
        n = w.shape[1]
        return np.ascontiguousarray(
            w.reshape(H, n // 128, 128)[:, :, perm].reshape(H, n)
        )

    def pack_w(w):
        # [H, n] -> [n/128 blk, 128 p, 32 ko, 128 c], contiguous per (blk, p)
        n = w.shape[1]
        return w.reshape(32, 128, n // 128, 128).transpose(2, 1, 0, 3)

    inv_freq = 1.0 / (ROPE_BASE ** (np.arange(0, D, 2, dtype=np.float32) / D))
    t = np.arange(sp, sp + S, dtype=np.float32)
    freqs = t[None, :] * inv_freq[:, None]  # [64, S]
    sin, cos = np.sin(freqs), np.cos(freqs)
    A = np.concatenate([sin, sin], axis=0).astype(np.float32)  # [128, S]
    Bm = np.concatenate([-cos, cos], axis=0).astype(np.float32)
    scale = np.float32(1.0 / np.sqrt(np.float32(D)))
    maps = {
        "ropeAq": _to_bf16(A * scale),
        "ropeBq": _to_bf16(Bm * scale),
        "ropeAk": _to_bf16(A),
        "ropeBk": _to_bf16(Bm),
    }

    in_maps = []
    for c in range(NCORES):
        b, j = divmod(c, 2)
        im = {
            "xT": _to_bf16(
                x[b].T.reshape(32, 128, 1024).transpose(1, 0, 2)
            ),
            "wq": _to_bf16(pack_w(permute_cols(wq[:, j * QC : (j + 1) * QC]))),
            "wk": _to_bf16(pack_w(permute_cols(wk[:, j * KC : (j + 1) * KC]))),
            "wv": _to_bf16(pack_w(wv[:, j * VC : (j + 1) * VC])),
            "wo": _to_bf16(
                wo[j * COH : (j + 1) * COH, :]
                .reshape(2, 8, 128, 8, 512)
                .transpose(0, 3, 2, 1, 4)
            ),
        }
        im.update(maps)
        in_maps.append(im)
    return in_maps


def kernel(x, wq, wk, wv, wo, start_pos=0, _trace=False):
    from concourse.bass_utils import run_bass_kernel_spmd

    if "nc" not in _CACHE:
        _CACHE["nc"] = _build()
    nc = _CACHE["nc"]

    in_maps = _host_prep(x, wq, wk, wv, wo, start_pos)
    res = run_bass_kernel_spmd(nc, in_maps, core_ids=list(range(NCORES)), trace=_trace)
    _CACHE["last_result"] = res

    out = np.empty((B, S, H), dtype=np.float32)
    for b in range(B):
        out[b] = res.results[2 * b]["out"] + res.results[2 * b + 1]["out"]
    return out



# revision 13
# speedup vs baseline: 1.3869x; 1.3869x over previous
"""GQA attention (B=4, S=1024, H=4096, 32 q heads / 8 kv heads, rotary) on 8 trn2 cores.

Sharding: DP4 x TP2. Core c = 2*b + j handles batch b with kv-head half j:
  - column-parallel wq/wk/wv (16 q heads / 4 kv heads per core)
  - row-parallel wo -> partial [S, H] outputs, host sums core pairs.

v3 dataflow (bf16 operands, fp32 PSUM/rope math, one K pass, no DRAM spills):
  xT resident in SBUF as bf16 [128, 32, 1024]; weights streamed bf16
  through a 3-buffer global rotation W[g % 3] whose global block index g
  runs across loop bodies AND For_i iterations (24 % 3 == 0 keeps the
  rotation seamless at every boundary), so the weight stream never
  restarts cold.  xt is a global tile reloaded for the next body 2
  chunks per phase-3 hh step, behind that body's wo strips in the DMA
  FIFO.  Projection blocks (24 x 128 cols) accumulate full K=4096 in
  PSUM, rope applied to an SBUF copy (DVE), pair-swap via SBUF DMA.
  Attention for q-block i is interleaved into the PE stream between
  projection blocks (scores -> ACT exp -> DVE-tree denominator reduce ->
  single sum-broadcast matmul -> DVE fast reciprocal -> attn@v -> DVE
  normalize into oT).  Phase 3: out = oT.T @ wo with wo streamed bf16.
"""

import numpy as np

B = 4
S = 1024
H = 4096
D = 128
HQ = 32
HKV = 8
G = 4
NCORES = 8
QC = 2048  # q cols per core
KC = 512  # k cols per core
VC = 512  # v cols per core
COH = 2048  # wo rows per core
ROPE_BASE = 10000.0

_CACHE = {}


def _build(reps=1, gpsimd_denom=False, hw_loop=False, staggered=False):
    import concourse.tile as tile
    from concourse import bacc, bass_isa, mybir
    from concourse.masks import make_identity

    fp32 = mybir.dt.float32
    bf16 = mybir.dt.bfloat16
    f32r = mybir.dt.float32r

    nc = bacc.Bacc(None, target_bir_lowering=False)

    # packed layouts: per-partition-contiguous so each DMA descriptor is
    # one 4-8 KiB run (256 B descriptors run far below HBM line rate)
    xT_d = nc.dram_tensor("xT", [128, H // 128, S], bf16, kind="ExternalInput")
    wq_d = nc.dram_tensor(
        "wq", [QC // 128, 128, H // 128, 128], bf16, kind="ExternalInput"
    )
    wk_d = nc.dram_tensor(
        "wk", [KC // 128, 128, H // 128, 128], bf16, kind="ExternalInput"
    )
    wv_d = nc.dram_tensor(
        "wv", [VC // 128, 128, H // 128, 128], bf16, kind="ExternalInput"
    )
    wo_d = nc.dram_tensor(
        "wo", [2, 8, 128, 8, 512], bf16, kind="ExternalInput"
    )
    aq_d = nc.dram_tensor("ropeAq", [D, S], bf16, kind="ExternalInput")
    bq_d = nc.dram_tensor("ropeBq", [D, S], bf16, kind="ExternalInput")
    ak_d = nc.dram_tensor("ropeAk", [D, S], bf16, kind="ExternalInput")
    bk_d = nc.dram_tensor("ropeBk", [D, S], bf16, kind="ExternalInput")
    out_d = nc.dram_tensor("out", [S, H], fp32, kind="ExternalOutput")

    out_r = out_d.rearrange("(tb p) h -> tb p h", p=128)  # [8, 128, 4096]

    NKO = H // 128  # 32 contraction tiles
    XCH = 2  # ko tiles per xt load chunk

    # projection block schedule: per kv head h: k, v, then 4 q blocks
    sched = []
    for h in range(4):
        sched.append(("k", h))
        sched.append(("v", h))
        for g in range(4):
            sched.append(("q", h * 4 + g))
    NSCHED = len(sched)  # 24

    def one_rep(tc, glob, u, nb, wrap):
        xt = glob["xt"]
        W = glob["W"]
        load_wt = glob["load_wt"]
        load_xt_chunks = glob["load_xt_chunks"]

        with (
            tc.tile_pool(name="persist", bufs=1) as persist,
            tc.tile_pool(name="konst", bufs=1) as konst,
            tc.tile_pool(name="wopre", bufs=1) as wopre,
        ):
            oT = persist.tile([128, 16, S], bf16)  # 32 KiB/part
            ident = konst.tile([128, 128], bf16)
            make_identity(nc, ident[:])
            if not gpsimd_denom:
                ones_f = konst.tile([128, 128], fp32)
                nc.vector.memset(ones_f[:], 1.0)
                ones = konst.tile([128, 128], f32r)
                nc.vector.tensor_copy(ones[:], ones_f[:])
            wot00 = wopre.tile([128, 8, 512], bf16, name="wot00")

            with (
                tc.tile_pool(name="kv", bufs=1) as kvpool,
                tc.tile_pool(name="maps", bufs=1) as mpool,
                tc.tile_pool(name="qt", bufs=2) as qpool,
                tc.tile_pool(name="ev", bufs=2) as epool,
                tc.tile_pool(name="vraw", bufs=2) as vrawpool,
                tc.tile_pool(name="ex", bufs=2) as expool,
                tc.tile_pool(name="den", bufs=2) as dpool,
                tc.tile_pool(
                    name="psp", bufs=(4 if gpsimd_denom else 3), space="PSUM"
                ) as psp,
                tc.tile_pool(name="pssc", bufs=2, space="PSUM") as pssc,
                tc.tile_pool(name="pso", bufs=1, space="PSUM") as psop,
                tc.tile_pool(
                    name="pst", bufs=(1 if gpsimd_denom else 2), space="PSUM"
                ) as pst,
            ):
                # per-body resident tensors
                kT = kvpool.tile([128, 4, S], bf16)  # 8 KiB/part
                v = kvpool.tile([128, 8, VC], bf16)  # 8 KiB/part

                # rope maps reloaded per body (posted at previous body's
                # end; needed a full block into this body)
                maps = {}
                for nm, dram in (
                    ("Aq", aq_d), ("Bq", bq_d), ("Ak", ak_d), ("Bk", bk_d)
                ):
                    mt = mpool.tile([128, S], bf16, name=nm)
                    nc.sync.dma_start(mt[:], dram[:])
                    maps[nm] = mt

                def rope(raw, Am, Bm, out_ap):
                    """out = A*raw + B*swap(raw), raw [128, S] bf16 in SBUF."""
                    sw = epool.tile([128, S], bf16, tag="sw", name="sw")
                    nc.sync.dma_start(sw[0:64, :], raw[64:128, :])
                    nc.sync.dma_start(sw[64:128, :], raw[0:64, :])
                    t1 = epool.tile([128, S], fp32, tag="t1", name="t1", bufs=1)
                    nc.vector.tensor_mul(t1[:], raw[:], Am[:])
                    t2 = epool.tile([128, S], fp32, tag="t2", name="t2", bufs=1)
                    nc.vector.tensor_mul(t2[:], sw[:], Bm[:])
                    nc.vector.tensor_add(out_ap, t1[:], t2[:])

                def proj_begin(i):
                    """A-half (t 0:512) of projection block i."""
                    gidx = u * NSCHED + i
                    wt = W[gidx % 3]
                    # prefetch two blocks ahead (possibly into the next
                    # body / next For_i iteration: same weights either way)
                    gp = gidx + 2
                    if gp < nb * NSCHED or wrap:
                        load_wt(gp)
                    raw = vrawpool.tile([128, S], bf16, tag="raw", name="raw")
                    psA = psp.tile([128, 512], fp32, tag="psp", name="psA")
                    for ko in range(NKO):
                        nc.tensor.matmul(
                            psA[:], wt[:, ko, :], xt[:, ko, 0:512],
                            start=(ko == 0), stop=(ko == NKO - 1),
                        )
                    nc.scalar.copy(raw[:, 0:512], psA[:])
                    return wt, raw

                def proj_end(ctx, i):
                    """B-half + rope/v epilogue; returns (cb, qt) for q blocks."""
                    kind, idx = sched[i]
                    wt, raw = ctx
                    psB = psp.tile([128, 512], fp32, tag="psp", name="psB")
                    for ko in range(NKO):
                        nc.tensor.matmul(
                            psB[:], wt[:, ko, :], xt[:, ko, 512:1024],
                            start=(ko == 0), stop=(ko == NKO - 1),
                        )
                    nc.scalar.copy(raw[:, 512:1024], psB[:])
                    if kind == "q":
                        qt = qpool.tile([128, S], bf16, tag="qt", name="qt")
                        rope(raw, maps["Aq"], maps["Bq"], qt[:])
                        return (idx, qt)
                    if kind == "k":
                        rope(raw, maps["Ak"], maps["Bk"], kT[:, idx, :])
                        return None
                    # v: PE-transpose [d, t] -> [t, d] blocks into natural v
                    for tb in range(8):
                        pt = pst.tile([128, 128], bf16, tag="misc", name="pt")
                        nc.tensor.transpose(
                            pt[:], raw[:, tb * 128 : (tb + 1) * 128], ident[:]
                        )
                        nc.vector.tensor_copy(
                            v[:, tb, idx * 128 : (idx + 1) * 128], pt[:]
                        )
                    return None

                def attn_scores(cb, qt, sh):
                    """scores + exp for (cb, sh): returns expT tile."""
                    h = cb // 4
                    ss = slice(sh * 512, sh * 512 + 512)
                    expT = expool.tile([128, 8, 512], bf16, tag="expT", name="expT")
                    for tb in range(8):
                        psc = pssc.tile([128, 512], fp32, tag="psc", name="psc")
                        nc.tensor.matmul(
                            psc[:],
                            kT[:, h, tb * 128 : (tb + 1) * 128],
                            qt[:, ss],
                            start=True, stop=True,
                        )
                        nc.scalar.activation(
                            expT[:, tb], psc[:],
                            mybir.ActivationFunctionType.Exp,
                        )
                    return expT

                def attn_tree(expT):
                    """DVE tree: pre-reduce 8 exp tiles to one [128, 512]."""
                    acc = dpool.tile([128, 512], f32r, tag="acc", name="acc",
                                     bufs=4)
                    nc.vector.tensor_add(acc[:], expT[:, 0], expT[:, 1])
                    for tb in range(2, 8):
                        nxt = dpool.tile([128, 512], f32r, tag="acc", name="acc",
                                         bufs=4)
                        nc.vector.tensor_add(nxt[:], acc[:], expT[:, tb])
                        acc = nxt
                    return acc[:]

                def attn_pden(g):
                    """cross-partition sum of acc, broadcast to all rows."""
                    if gpsimd_denom:
                        red = dpool.tile([128, 512], fp32, tag="acc", name="red",
                                         bufs=4)
                        nc.gpsimd.partition_all_reduce(
                            red[:], g["acc"], 128, bass_isa.ReduceOp.add
                        )
                        inv = dpool.tile([128, 512], fp32, tag="inv", name="inv")
                        nc.vector.reciprocal_approx_fast(inv[:], red[:])
                        g["inv"] = inv[:]
                        return
                    # ones[128,128].T @ acc = column sums broadcast to all
                    # 128 partitions in a single matmul
                    pbc = pst.tile([128, 512], fp32, tag="misc", name="pbc")
                    nc.tensor.matmul(
                        pbc[:], ones[:, :], g["acc"], start=True, stop=True
                    )
                    g["den_ps"] = pbc[:]

                def attn_pbc(g):
                    """1/den straight from PSUM via fast Newton reciprocal."""
                    if gpsimd_denom:
                        return
                    inv = dpool.tile([128, 512], fp32, tag="inv", name="inv")
                    nc.vector.reciprocal_approx_fast(inv[:], g["den_ps"])
                    g["inv"] = inv[:]

                def attn_av(g):
                    cb, sh, expT = g["cb"], g["sh"], g["expT"]
                    h = cb // 4
                    ss = slice(sh * 512, sh * 512 + 512)
                    po = psop.tile([128, 512], fp32, tag="po", name="po")
                    for tb in range(8):
                        nc.tensor.matmul(
                            po[:],
                            v[:, tb, h * 128 : (h + 1) * 128],
                            expT[:, tb],
                            start=(tb == 0), stop=(tb == 7),
                        )
                    nc.vector.tensor_mul(oT[:, cb, ss], po[:], g["inv"])

                # software pipeline. Iteration i issues, in PE order:
                #   projA(i) | pden(G_{i-2}) | projB(i)+epilogue | pbc+attnv
                #   (G_{i-2}) | scores+exp+tree (G_{i-1})
                # so every cross-engine chain (exp on ACT, tree on DVE,
                # reciprocal) has a full projection block of PE time to hide.
                sc_q = []  # (cb, qt): q blocks awaiting scores
                s2_q = []  # groups awaiting pden
                for i in range(NSCHED + 2):
                    ctx = proj_begin(i) if i < NSCHED else None
                    s3_q = []
                    while s2_q:
                        g = s2_q.pop(0)
                        attn_pden(g)
                        s3_q.append(g)
                    new_q = proj_end(ctx, i) if ctx is not None else None
                    while s3_q:
                        g = s3_q.pop(0)
                        attn_pbc(g)
                        attn_av(g)
                    while sc_q:
                        cb0, qt0 = sc_q.pop(0)
                        for sh in range(2):
                            e = attn_scores(cb0, qt0, sh)
                            acc = attn_tree(e)
                            s2_q.append(
                                {"cb": cb0, "sh": sh, "expT": e, "acc": acc}
                            )
                    if new_q is not None:
                        sc_q.append(new_q)

                # prefetch first wo strip while attention tail drains
                nc.sync.dma_start(wot00[:], wo_d[0, 0])

            # ---------------- Phase 3: out = oT.T @ wo ----------------
            # the next body's xt reload rides along 2 chunks per hh step,
            # always behind this body's wo strip loads in the DMA FIFO
            reload_xt = u < nb - 1 or wrap
            with (
                tc.tile_pool(name="wot", bufs=4) as wopool,
                tc.tile_pool(name="outp", bufs=3) as outpool,
                tc.tile_pool(name="psout", bufs=3, space="PSUM") as psout,
            ):
                def load_wo_strip(hh, hf):
                    wot = wopool.tile(
                        [128, 8, 512], bf16, tag="wo", name=f"wo{hf}"
                    )
                    nc.sync.dma_start(wot[:], wo_d[hf, hh])
                    return wot

                def p3_mm(pso_, tb, co, wotA, wotB, start=False, stop=False):
                    wot = wotA if co < 8 else wotB
                    nc.tensor.matmul(
                        pso_[:],
                        oT[:, co, tb * 128 : (tb + 1) * 128],
                        wot[:, co % 8, :],
                        start=start, stop=stop,
                    )

                def p3_close(pso_, tb, hh):
                    ot = outpool.tile([128, 512], fp32, tag="ot", name="ot")
                    nc.scalar.copy(ot[:], pso_[:])
                    nc.sync.dma_start(
                        out_r[tb, :, hh * 512 : hh * 512 + 512], ot[:]
                    )

                wo_next = [wot00, load_wo_strip(0, 1)]
                # hh=0 streams in two passes: co 0..11 for three tb groups
                # first (ready early), deferring co 12..15 -- whose oT
                # columns the attention tail is still writing -- so the PE
                # keeps streaming while the drain finishes
                wotA, wotB = wo_next
                g3 = []
                for tb in range(3):
                    pso_ = psout.tile([128, 512], fp32, tag="pso", name="pso_")
                    for co in range(12):
                        p3_mm(pso_, tb, co, wotA, wotB, start=(co == 0))
                    g3.append(pso_)
                for tb in range(3):
                    pso_ = g3[tb]
                    for co in range(12, 16):
                        p3_mm(pso_, tb, co, wotA, wotB, stop=(co == 15))
                    if tb == 0:
                        wo_next[0] = load_wo_strip(1, 0)
                    elif tb == 1:
                        wo_next[1] = load_wo_strip(1, 1)
                    p3_close(pso_, tb, 0)
                for tb in range(3, 8):
                    pso_ = psout.tile([128, 512], fp32, tag="pso", name="pso_")
                    for co in range(16):
                        p3_mm(pso_, tb, co, wotA, wotB,
                              start=(co == 0), stop=(co == 15))
                    if tb == 3 and reload_xt:
                        load_xt_chunks(0, 2)
                    p3_close(pso_, tb, 0)
                for hh in range(1, 8):
                    wotA, wotB = wo_next
                    for tb in range(8):
                        pso_ = psout.tile([128, 512], fp32, tag="pso", name="pso_")
                        for co in range(8):
                            p3_mm(pso_, tb, co, wotA, wotB, start=(co == 0))
                        if hh < 7:
                            if tb == 0:
                                wo_next[0] = load_wo_strip(hh + 1, 0)
                            elif tb == 1:
                                wo_next[1] = load_wo_strip(hh + 1, 1)
                        if tb == 2 and reload_xt:
                            load_xt_chunks(2 * hh, 2 * hh + 2)
                        for co in range(8, 16):
                            p3_mm(pso_, tb, co, wotA, wotB, stop=(co == 15))
                        p3_close(pso_, tb, hh)

    def body_seq(tc, glob, nb, wrap):
        for u in range(nb):
            one_rep(tc, glob, u, nb, wrap)

    with tile.TileContext(nc) as tc, nc.allow_low_precision(
        reason="bf16 matmul pipeline, fp32 accumulation"
    ):
        with tc.tile_pool(name="glob", bufs=1) as globpool:
            xt = globpool.tile([128, NKO, S], bf16, name="xt")  # 64 KiB/part
            W = [
                globpool.tile([128, NKO, 128], bf16, name=f"W{j}")
                for j in range(3)
            ]

            def load_wt(gidx, split=1):
                kind, idx = sched[gidx % NSCHED]
                w_d = {"q": wq_d, "k": wk_d, "v": wv_d}[kind]
                wt = W[gidx % 3]
                step = NKO // split
                for s in range(split):
                    ks = slice(s * step, (s + 1) * step)
                    nc.sync.dma_start(wt[:, ks, :], w_d[idx][:, ks, :])
                return wt

            def load_xt_chunks(lo, hi):
                for chx in range(lo, hi):
                    nc.sync.dma_start(
                        xt[:, chx * XCH : (chx + 1) * XCH, :],
                        xT_d[:, chx * XCH : (chx + 1) * XCH, :],
                    )

            glob = {
                "xt": xt,
                "W": W,
                "load_wt": load_wt,
                "load_xt_chunks": load_xt_chunks,
            }

            # cold-start loads: block 0 split 4-ways so the PE can start
            # on ko 0-7 while the rest streams in; then xt, then block 1
            load_wt(0, split=4)
            load_xt_chunks(0, 4)
            load_wt(1)
            load_xt_chunks(4, NKO // XCH)

            if hw_loop and reps > 1:
                unroll = 1
                while reps % (unroll * 2) == 0 and unroll < hw_loop:
                    unroll *= 2
                with tc.For_i(0, reps // unroll, 1, staggered_reset=staggered):
                    body_seq(tc, glob, unroll, wrap=True)
            else:
                body_seq(tc, glob, reps, wrap=False)

    nc.compile()
    return nc


def _to_bf16(a):
    import ml_dtypes

    return np.asarray(a, dtype=np.float32).astype(ml_dtypes.bfloat16)


def _host_prep(x, wq, wk, wv, wo, start_pos):
    x = np.asarray(x, dtype=np.float32)
    wq = np.asarray(wq, dtype=np.float32)
    wk = np.asarray(wk, dtype=np.float32)
    wv = np.asarray(wv, dtype=np.float32)
    wo = np.asarray(wo, dtype=np.float32)
    sp = int(np.asarray(start_pos))

    perm = np.concatenate([np.arange(0, 128, 2), np.arange(1, 128, 2)])

    def permute_cols(w):
        n = w.shape[1]
        return np.ascontiguousarray(
            w.reshape(H, n // 128, 128)[:, :, perm].reshape(H, n)
        )

    def pack_w(w):
        # [H, n] -> [n/128 blk, 128 p, 32 ko, 128 c], contiguous per (blk, p)
        n = w.shape[1]
        return w.reshape(32, 128, n // 128, 128).transpose(2, 1, 0, 3)

    inv_freq = 1.0 / (ROPE_BASE ** (np.arange(0, D, 2, dtype=np.float32) / D))
    t = np.arange(sp, sp + S, dtype=np.float32)
    freqs = t[None, :] * inv_freq[:, None]  # [64, S]
    sin, cos = np.sin(freqs), np.cos(freqs)
    A = np.concatenate([sin, sin], axis=0).astype(np.float32)  # [128, S]
    Bm = np.concatenate([-cos, cos], axis=0).astype(np.float32)
    scale = np.float32(1.0 / np.sqrt(np.float32(D)))
    maps = {
        "ropeAq": _to_bf16(A * scale),
        "ropeBq": _to_bf16(Bm * scale),
        "ropeAk": _to_bf16(A),
        "ropeBk": _to_bf16(Bm),
    }

    in_maps = []
    for c in range(NCORES):
        b, j = divmod(c, 2)
        im = {
            "xT": _to_bf16(
                x[b].T.reshape(32, 128, 1024).transpose(1, 0, 2)
            ),
            "wq": _to_bf16(pack_w(permute_cols(wq[:, j * QC : (j + 1) * QC]))),
            "wk": _to_bf16(pack_w(permute_cols(wk[:, j * KC : (j + 1) * KC]))),
            "wv": _to_bf16(pack_w(wv[:, j * VC : (j + 1) * VC])),
            "wo": _to_bf16(
                wo[j * COH : (j + 1) * COH, :]
                .reshape(2, 8, 128, 8, 512)
                .transpose(0, 3, 2, 1, 4)
            ),
        }
        im.update(maps)
        in_maps.append(im)
    return in_maps


def kernel(x, wq, wk, wv, wo, start_pos=0, _trace=False):
    from concourse.bass_utils import run_bass_kernel_spmd

    if "nc" not in _CACHE:
        _CACHE["nc"] = _build()
    nc = _CACHE["nc"]

    in_maps = _host_prep(x, wq, wk, wv, wo, start_pos)
    res = run_bass_kernel_spmd(nc, in_maps, core_ids=list(range(NCORES)), trace=_trace)
    _CACHE["last_result"] = res

    out = np.empty((B, S, H), dtype=np.float32)
    for b in range(B):
        out[b] = res.results[2 * b]["out"] + res.results[2 * b + 1]["out"]
    return out
